# revision 3
# baseline (speedup 1.0000x reference)
"""Trainium2 Bass kernel for nn_BagKQMClassModel.

Computation (per batch item b):
    K[b,n,m]   = exp(-d2/(2 s^2)),  d2 = |A[b,n] - C[m]|^2
    out_w[b,m] = (1/N) sum_n comp_w[m] * K^2
    y_w        = out_w / sum_m out_w
    probs      = y_w @ (y_v^2),  y_v = c_y rows normalized

Key transformations used here:
  * K^2 = exp(-d2/s^2), so only one exp per (b,n,m) element is needed.
  * d2 = a2[bn] + b2[m] - 2 g[m,bn] with g = C @ A^T.  Both -a2/2 and -b2/2
    are folded into the matmul by augmenting the contraction dim (K=34):
        row 32: C^T row = 1,       A^T row = -a2/2
        row 33: C^T row = -b2/2,   A^T row = 1
    so one matmul emits g_full with exp argument = (2/s^2) * g_full and the
    activation needs no bias at all.
  * probs = T[:, :10] / T[:, 10] where T[b,:] = sum_n sum_m K2[m,bn]*W[m,:],
    W[m, :10] = comp_w[m] * c_y[m]^2 / |c_y[m]|^2,  W[m, 10] = comp_w[m].
    The 1/N bag weight and normalization cancel.
  * Layout (m on partitions, b*n on free dim): matmul2 contracts m on the PE
    with W as the stationary operand; the final n-reduction happens on the
    tiny (11, bn) result via a segmented DVE reduce.

Sharding: batch 256 -> 32 items per core across 8 cores; c_x/c_y/comp_w
replicated. No collectives (forward only).
"""

import numpy as np

import concourse.bacc as bacc
import concourse.mybir as mybir
import concourse.tile as tile
from concourse.bass import ts
from concourse.bass_utils import run_bass_kernel_spmd
from concourse.masks import make_identity

NCORES = 8
BS, N, DX, DY, M = 256, 128, 32, 10, 2048
BPC = BS // NCORES      # 32 batch items per core
MB = M // 128           # 16 chunks of the component axis
KAUG = DX + 2           # 34: contraction dim with the two folded rows
NBLK = 4                # bn blocks per core
BLKI = BPC // NBLK      # 8 items per block
F_BLK = BLKI * N        # 1024 free elements per (block, m-chunk) tile
MIN_SIGMA = 1e-3
FP32 = mybir.dt.float32
AX = mybir.AxisListType
ALU = mybir.AluOpType
ACTF = mybir.ActivationFunctionType


def _body(tc, inp, cx, cy, cw_d, out_d, scale):
    nc = tc.nc
    from contextlib import ExitStack

    with ExitStack() as ctx:
        const = ctx.enter_context(tc.tile_pool(name="const", bufs=1))
        work = ctx.enter_context(tc.tile_pool(name="work", bufs=2))
        k2p = ctx.enter_context(tc.tile_pool(name="k2p", bufs=3))
        psum = ctx.enter_context(tc.tile_pool(name="psum", bufs=2, space="PSUM"))

        identity = const.tile([128, 128], FP32)
        make_identity(nc, identity)

        # ---- load replicated constants -----------------------------------
        # c_x goes into an augmented (128, 34) layout per chunk: cols 0:32 are
        # c_x, col 32 = 1.0, col 33 = -b2/2.  Transposing the whole tile then
        # yields the augmented C^T rows without any partition-offset writes.
        cx_aug = const.tile([128, MB, KAUG], FP32)
        nc.sync.dma_start(
            out=cx_aug[:, :, 0:DX], in_=cx.rearrange("(t p) d -> p t d", p=128)
        )
        nc.vector.memset(cx_aug[:, :, DX : DX + 1], 1.0)
        cy_all = const.tile([128, MB, DY], FP32)
        nc.sync.dma_start(out=cy_all, in_=cy.rearrange("(t p) d -> p t d", p=128))
        cw_sb = const.tile([128, MB], FP32)
        nc.sync.dma_start(out=cw_sb, in_=cw_d.rearrange("(t p) -> p t", p=128))

        # ---- load batch shard, leaving room for the two augmented cols ----
        A_aug = const.tile([128, BPC, KAUG], FP32)
        nc.sync.dma_start(
            out=A_aug[:, :, 0:DX], in_=inp.rearrange("t p d -> p t d")
        )
        nc.vector.memset(A_aug[:, :, DX + 1 : DX + 2], 1.0)

        CT = const.tile([KAUG, M], FP32)      # augmented C^T (stationary mm1)
        AT = const.tile([KAUG, BPC, N], FP32)  # augmented A^T (moving mm1)
        W_all = const.tile([128, MB, DY + 1], FP32)
        T_sb = const.tile([DY + 1, BPC], FP32)

        # ---- per m-chunk setup: C^T, b2, W -------------------------------
        for mb in range(MB):
            sq = work.tile([128, DX], FP32, tag="sq")
            nc.vector.tensor_mul(sq, cx_aug[:, mb, 0:DX], cx_aug[:, mb, 0:DX])
            nc.vector.tensor_reduce(
                out=cx_aug[:, mb, DX + 1 : DX + 2], in_=sq, axis=AX.X, op=ALU.add
            )
            nc.vector.tensor_scalar_mul(
                cx_aug[:, mb, DX + 1 : DX + 2], cx_aug[:, mb, DX + 1 : DX + 2], -0.5
            )
            trc = psum.tile([KAUG, 128], FP32, tag="g")
            nc.tensor.transpose(trc, cx_aug[:, mb, :], identity)
            nc.vector.tensor_copy(CT[:, ts(mb, 128)], trc)

            sqy = work.tile([128, DY], FP32, tag="sqy")
            nc.vector.tensor_mul(sqy, cy_all[:, mb, :], cy_all[:, mb, :])
            ssum = work.tile([128, 1], FP32, tag="ssum")
            nc.vector.tensor_reduce(out=ssum, in_=sqy, axis=AX.X, op=ALU.add)
            rec = work.tile([128, 1], FP32, tag="rec")
            nc.vector.reciprocal(rec, ssum)
            facr = work.tile([128, 1], FP32, tag="facr")
            nc.vector.tensor_mul(facr, rec, cw_sb[:, mb : mb + 1])
            nc.vector.tensor_scalar(
                out=W_all[:, mb, 0:DY],
                in0=sqy,
                scalar1=facr,
                scalar2=None,
                op0=ALU.mult,
            )
            nc.vector.tensor_copy(W_all[:, mb, DY : DY + 1], cw_sb[:, mb : mb + 1])

        # ---- per item: -a2/2 column, transpose into AT -------------------
        for i in range(BPC):
            sqa = work.tile([128, DX], FP32, tag="sq")
            nc.vector.tensor_mul(sqa, A_aug[:, i, 0:DX], A_aug[:, i, 0:DX])
            nc.vector.tensor_reduce(
                out=A_aug[:, i, DX : DX + 1], in_=sqa, axis=AX.X, op=ALU.add
            )
            nc.vector.tensor_scalar_mul(
                A_aug[:, i, DX : DX + 1], A_aug[:, i, DX : DX + 1], -0.5
            )
            tra = psum.tile([KAUG, 128], FP32, tag="g")
            nc.tensor.transpose(tra, A_aug[:, i, :], identity)
            nc.vector.tensor_copy(AT[:, i, :], tra)

        # ---- main pipeline ------------------------------------------------
        for blk in range(NBLK):
            S = psum.tile([DY + 1, F_BLK], FP32, tag="s")
            for mb in range(MB):
                g = psum.tile([128, F_BLK], FP32, tag="g")
                for q in range(F_BLK // 512):
                    nc.tensor.matmul(
                        g[:, ts(q, 512)],
                        CT[:, ts(mb, 128)],
                        AT[:, blk * BLKI + q * 4 : blk * BLKI + (q + 1) * 4, :],
                        start=True,
                        stop=True,
                    )
                K2 = k2p.tile([128, F_BLK], FP32, tag="k2")
                nc.scalar.activation(K2, g, ACTF.Exp, bias=0.0, scale=scale)
                for q in range(F_BLK // 512):
                    nc.tensor.matmul(
                        S[:, ts(q, 512)],
                        W_all[:, mb, :],
                        K2[:, ts(q, 512)],
                        start=(mb == 0),
                        stop=(mb == MB - 1),
                    )
            nc.vector.tensor_reduce(
                out=T_sb[:, blk * BLKI : (blk + 1) * BLKI],
                in_=S.rearrange("p (t n) -> p t n", n=N),
                axis=AX.X,
                op=ALU.add,
            )

        # ---- epilogue: probs = T[:, :10] / T[:, 10] -----------------------
        trT = psum.tile([BPC, DY + 1], FP32, tag="g")
        nc.tensor.transpose(trT, T_sb, identity[0 : DY + 1, 0 : DY + 1])
        Tt = const.tile([BPC, DY + 1], FP32)
        nc.vector.tensor_copy(Tt, trT)
        recd = const.tile([BPC, 1], FP32)
        nc.vector.reciprocal(recd, Tt[:, DY : DY + 1])
        outsb = const.tile([BPC, DY], FP32)
        nc.vector.tensor_scalar(
            out=outsb, in0=Tt[:, 0:DY], scalar1=recd, scalar2=None, op0=ALU.mult
        )
        nc.sync.dma_start(out=out_d, in_=outsb)


def build_program(scale):
    nc = bacc.Bacc(
        "TRN2",
        target_bir_lowering=False,
        debug=False,
        enable_asserts=False,
        num_devices=NCORES,
    )
    inp = nc.dram_tensor("inputs", [BPC, N, DX], FP32, kind="ExternalInput").ap()
    cx = nc.dram_tensor("c_x", [M, DX], FP32, kind="ExternalInput").ap()
    cy = nc.dram_tensor("c_y", [M, DY], FP32, kind="ExternalInput").ap()
    cw = nc.dram_tensor("comp_w", [M], FP32, kind="ExternalInput").ap()
    out = nc.dram_tensor("out", [BPC, DY], FP32, kind="ExternalOutput").ap()
    with tile.TileContext(nc) as tc:
        _body(tc, inp, cx, cy, cw, out, scale)
    nc.compile()
    return nc


_PROGRAM_CACHE: dict = {}


def _get_program(scale):
    nc = _PROGRAM_CACHE.get(scale)
    if nc is None:
        nc = build_program(scale)
        _PROGRAM_CACHE[scale] = nc
    return nc


def make_in_maps(inputs, c_x, c_y, comp_w):
    shards = np.ascontiguousarray(inputs.reshape(NCORES, BPC, N, DX))
    return [
        {
            "inputs": shards[i],
            "c_x": np.ascontiguousarray(c_x),
            "c_y": np.ascontiguousarray(c_y),
            "comp_w": np.ascontiguousarray(comp_w),
        }
        for i in range(NCORES)
    ]


def scale_from_sigma(sigma) -> float:
    s = max(float(np.asarray(sigma, dtype=np.float64)), MIN_SIGMA)
    return float(2.0 / (s * s))


def kernel(inputs, sigma, c_x, c_y, comp_w, _run_kwargs=None):
    nc = _get_program(scale_from_sigma(sigma))
    in_maps = make_in_maps(inputs, c_x, c_y, comp_w)
    res = run_bass_kernel_spmd(
        nc, in_maps, core_ids=list(range(NCORES)), **(_run_kwargs or {})
    )
    out = np.concatenate([res.results[i]["out"] for i in range(NCORES)], axis=0)
    return out.astype(np.float32)


# revision 7
# speedup vs baseline: 1.6843x; 1.6843x over previous
"""Trainium2 Bass kernel for nn_BagKQMClassModel.

Computation (per batch item b):
    K[b,n,m]   = exp(-d2/(2 s^2)),  d2 = |A[b,n] - C[m]|^2
    out_w[b,m] = (1/N) sum_n comp_w[m] * K^2
    y_w        = out_w / sum_m out_w
    probs      = y_w @ (y_v^2),  y_v = c_y rows normalized

Key transformations used here:
  * K^2 = exp(-d2/s^2), so only one exp per (b,n,m) element is needed.
  * d2 = a2[bn] + b2[m] - 2 g[m,bn] with g = C @ A^T.  Both -a2/2 and -b2/2
    are folded into the matmul by augmenting the contraction dim (K=34):
        row 32: C^T row = 1,       A^T row = -a2/2
        row 33: C^T row = -b2/2,   A^T row = 1
    so one matmul emits g_full with exp argument = (2/s^2) * g_full and the
    activation needs no bias at all.
  * probs = T[:, :10] / T[:, 10] where T[b,:] = sum_n sum_m K2[m,bn]*W[m,:],
    W[m, :10] = comp_w[m] * c_y[m]^2 / |c_y[m]|^2,  W[m, 10] = comp_w[m].
    The 1/N bag weight and normalization cancel.
  * Layout (m on partitions, b*n on free dim): matmul2 contracts m on the PE
    with W as the stationary operand; the final n-reduction happens on the
    tiny (11, bn) result via a segmented DVE reduce.

Sharding: batch 256 -> 32 items per core across 8 cores; c_x/c_y/comp_w
replicated. No collectives (forward only).
"""

import numpy as np

import concourse.bacc as bacc
import concourse.mybir as mybir
import concourse.tile as tile
from concourse.bass import ts
from concourse.bass_utils import run_bass_kernel_spmd
from concourse.masks import make_identity

NCORES = 8
BS, N, DX, DY, M = 256, 128, 32, 10, 2048
BPC = BS // NCORES      # 32 batch items per core
MB = M // 128           # 16 chunks of the component axis
KAUG = DX + 2           # 34: contraction dim with the two folded rows
NBLK = 4                # bn blocks per core
BLKI = BPC // NBLK      # 8 items per block
F_BLK = BLKI * N        # 1024 free elements per (block, m-chunk) tile
MIN_SIGMA = 1e-3
FP32 = mybir.dt.float32
BF16 = mybir.dt.bfloat16
AX = mybir.AxisListType
ALU = mybir.AluOpType
ACTF = mybir.ActivationFunctionType


def _body(tc, inp, cx, cy, cw_d, out_d, scale):
    nc = tc.nc
    from contextlib import ExitStack

    with ExitStack() as ctx:
        const = ctx.enter_context(tc.tile_pool(name="const", bufs=1))
        work = ctx.enter_context(tc.tile_pool(name="work", bufs=2))
        k2p = ctx.enter_context(tc.tile_pool(name="k2p", bufs=3))
        psum = ctx.enter_context(tc.tile_pool(name="psum", bufs=2, space="PSUM"))

        identity = const.tile([128, 128], FP32)
        make_identity(nc, identity)

        # ---- load replicated constants -----------------------------------
        # c_x goes into an augmented (128, 34) layout per chunk: cols 0:32 are
        # c_x, col 32 = 1.0, col 33 = -b2/2.  Transposing the whole tile then
        # yields the augmented C^T rows without any partition-offset writes.
        cx_aug = const.tile([128, MB, KAUG], FP32)
        nc.sync.dma_start(
            out=cx_aug[:, :, 0:DX], in_=cx.rearrange("(t p) d -> p t d", p=128)
        )
        nc.vector.memset(cx_aug[:, :, DX : DX + 1], 1.0)
        cy_all = const.tile([128, MB, DY], FP32)
        nc.sync.dma_start(out=cy_all, in_=cy.rearrange("(t p) d -> p t d", p=128))
        cw_sb = const.tile([128, MB], FP32)
        nc.sync.dma_start(out=cw_sb, in_=cw_d.rearrange("(t p) -> p t", p=128))

        # ---- load batch shard, leaving room for the two augmented cols ----
        A_aug = const.tile([128, BPC, KAUG], FP32)
        nc.sync.dma_start(
            out=A_aug[:, :, 0:DX], in_=inp.rearrange("t p d -> p t d")
        )
        nc.vector.memset(A_aug[:, :, DX + 1 : DX + 2], 1.0)

        # bf16 for both matmul operands: fp32 matmuls stream at 1/4 PE rate.
        CT = const.tile([KAUG, M], BF16)      # augmented C^T (stationary mm1)
        AT = const.tile([KAUG, BPC, N], BF16)  # augmented A^T (moving mm1)
        W_all = const.tile([128, MB, DY + 1], FP32)
        W_bf = const.tile([128, MB, DY + 1], BF16)
        T_sb = const.tile([DY + 1, BPC], FP32)

        # ---- per m-chunk setup: C^T, b2, W -------------------------------
        for mb in range(MB):
            sq = work.tile([128, DX], FP32, tag="sq")
            nc.vector.tensor_mul(sq, cx_aug[:, mb, 0:DX], cx_aug[:, mb, 0:DX])
            nc.vector.tensor_reduce(
                out=cx_aug[:, mb, DX + 1 : DX + 2], in_=sq, axis=AX.X, op=ALU.add
            )
            nc.vector.tensor_scalar_mul(
                cx_aug[:, mb, DX + 1 : DX + 2], cx_aug[:, mb, DX + 1 : DX + 2], -0.5
            )
            trc = psum.tile([KAUG, 128], FP32, tag="g")
            nc.tensor.transpose(trc, cx_aug[:, mb, :], identity)
            nc.vector.tensor_copy(CT[:, ts(mb, 128)], trc)

            sqy = work.tile([128, DY], FP32, tag="sqy")
            nc.vector.tensor_mul(sqy, cy_all[:, mb, :], cy_all[:, mb, :])
            ssum = work.tile([128, 1], FP32, tag="ssum")
            nc.vector.tensor_reduce(out=ssum, in_=sqy, axis=AX.X, op=ALU.add)
            rec = work.tile([128, 1], FP32, tag="rec")
            nc.vector.reciprocal(rec, ssum)
            facr = work.tile([128, 1], FP32, tag="facr")
            nc.vector.tensor_mul(facr, rec, cw_sb[:, mb : mb + 1])
            nc.vector.tensor_scalar(
                out=W_all[:, mb, 0:DY],
                in0=sqy,
                scalar1=facr,
                scalar2=None,
                op0=ALU.mult,
            )
            nc.vector.tensor_copy(W_all[:, mb, DY : DY + 1], cw_sb[:, mb : mb + 1])
        nc.vector.tensor_copy(W_bf, W_all)

        # ---- per item: -a2/2 column, transpose into AT -------------------
        for i in range(BPC):
            sqa = work.tile([128, DX], FP32, tag="sq")
            nc.vector.tensor_mul(sqa, A_aug[:, i, 0:DX], A_aug[:, i, 0:DX])
            nc.vector.tensor_reduce(
                out=A_aug[:, i, DX : DX + 1], in_=sqa, axis=AX.X, op=ALU.add
            )
            nc.vector.tensor_scalar_mul(
                A_aug[:, i, DX : DX + 1], A_aug[:, i, DX : DX + 1], -0.5
            )
            tra = psum.tile([KAUG, 128], FP32, tag="g")
            nc.tensor.transpose(tra, A_aug[:, i, :], identity)
            nc.vector.tensor_copy(AT[:, i, :], tra)

        # ---- main pipeline ------------------------------------------------
        for blk in range(NBLK):
            S = psum.tile([DY + 1, F_BLK], FP32, tag="s")
            for mb in range(MB):
                g = psum.tile([128, F_BLK], FP32, tag="g")
                for q in range(F_BLK // 512):
                    nc.tensor.matmul(
                        g[:, ts(q, 512)],
                        CT[:, ts(mb, 128)],
                        AT[:, blk * BLKI + q * 4 : blk * BLKI + (q + 1) * 4, :],
                        start=True,
                        stop=True,
                    )
                K2 = k2p.tile([128, F_BLK], BF16, tag="k2")
                nc.scalar.activation(K2, g, ACTF.Exp, bias=0.0, scale=scale)
                for q in range(F_BLK // 512):
                    nc.tensor.matmul(
                        S[:, ts(q, 512)],
                        W_bf[:, mb, :],
                        K2[:, ts(q, 512)],
                        start=(mb == 0),
                        stop=(mb == MB - 1),
                    )
            nc.vector.tensor_reduce(
                out=T_sb[:, blk * BLKI : (blk + 1) * BLKI],
                in_=S.rearrange("p (t n) -> p t n", n=N),
                axis=AX.X,
                op=ALU.add,
            )

        # ---- epilogue: probs = T[:, :10] / T[:, 10] -----------------------
        trT = psum.tile([BPC, DY + 1], FP32, tag="g")
        nc.tensor.transpose(trT, T_sb, identity[0 : DY + 1, 0 : DY + 1])
        Tt = const.tile([BPC, DY + 1], FP32)
        nc.vector.tensor_copy(Tt, trT)
        recd = const.tile([BPC, 1], FP32)
        nc.vector.reciprocal(recd, Tt[:, DY : DY + 1])
        outsb = const.tile([BPC, DY], FP32)
        nc.vector.tensor_scalar(
            out=outsb, in0=Tt[:, 0:DY], scalar1=recd, scalar2=None, op0=ALU.mult
        )
        nc.sync.dma_start(out=out_d, in_=outsb)


def build_program(scale):
    nc = bacc.Bacc(
        "TRN2",
        target_bir_lowering=False,
        debug=False,
        enable_asserts=False,
        num_devices=NCORES,
    )
    inp = nc.dram_tensor("inputs", [BPC, N, DX], FP32, kind="ExternalInput").ap()
    cx = nc.dram_tensor("c_x", [M, DX], FP32, kind="ExternalInput").ap()
    cy = nc.dram_tensor("c_y", [M, DY], FP32, kind="ExternalInput").ap()
    cw = nc.dram_tensor("comp_w", [M], FP32, kind="ExternalInput").ap()
    out = nc.dram_tensor("out", [BPC, DY], FP32, kind="ExternalOutput").ap()
    with tile.TileContext(nc) as tc:
        _body(tc, inp, cx, cy, cw, out, scale)
    nc.compile()
    return nc


_PROGRAM_CACHE: dict = {}


def _get_program(scale):
    nc = _PROGRAM_CACHE.get(scale)
    if nc is None:
        nc = build_program(scale)
        _PROGRAM_CACHE[scale] = nc
    return nc


def make_in_maps(inputs, c_x, c_y, comp_w):
    shards = np.ascontiguousarray(inputs.reshape(NCORES, BPC, N, DX))
    return [
        {
            "inputs": shards[i],
            "c_x": np.ascontiguousarray(c_x),
            "c_y": np.ascontiguousarray(c_y),
            "comp_w": np.ascontiguousarray(comp_w),
        }
        for i in range(NCORES)
    ]


def scale_from_sigma(sigma) -> float:
    s = max(float(np.asarray(sigma, dtype=np.float64)), MIN_SIGMA)
    return float(2.0 / (s * s))


def kernel(inputs, sigma, c_x, c_y, comp_w, _run_kwargs=None):
    nc = _get_program(scale_from_sigma(sigma))
    in_maps = make_in_maps(inputs, c_x, c_y, comp_w)
    res = run_bass_kernel_spmd(
        nc, in_maps, core_ids=list(range(NCORES)), **(_run_kwargs or {})
    )
    out = np.concatenate([res.results[i]["out"] for i in range(NCORES)], axis=0)
    return out.astype(np.float32)


# revision 8
# speedup vs baseline: 1.6869x; 1.0015x over previous
"""Trainium2 Bass kernel for nn_BagKQMClassModel.

Computation (per batch item b):
    K[b,n,m]   = exp(-d2/(2 s^2)),  d2 = |A[b,n] - C[m]|^2
    out_w[b,m] = (1/N) sum_n comp_w[m] * K^2
    y_w        = out_w / sum_m out_w
    probs      = y_w @ (y_v^2),  y_v = c_y rows normalized

Key transformations used here:
  * K^2 = exp(-d2/s^2), so only one exp per (b,n,m) element is needed.
  * d2 = a2[bn] + b2[m] - 2 g[m,bn] with g = C @ A^T.  Both -a2/2 and -b2/2
    are folded into the matmul by augmenting the contraction dim (K=34):
        row 32: C^T row = 1,       A^T row = -a2/2
        row 33: C^T row = -b2/2,   A^T row = 1
    so one matmul emits g_full with exp argument = (2/s^2) * g_full and the
    activation needs no bias at all.
  * probs = T[:, :10] / T[:, 10] where T[b,:] = sum_n sum_m K2[m,bn]*W[m,:],
    W[m, :10] = comp_w[m] * c_y[m]^2 / |c_y[m]|^2,  W[m, 10] = comp_w[m].
    The 1/N bag weight and normalization cancel.
  * Layout (m on partitions, b*n on free dim): matmul2 contracts m on the PE
    with W as the stationary operand; the final n-reduction happens on the
    tiny (11, bn) result via a segmented DVE reduce.

Sharding: batch 256 -> 32 items per core across 8 cores; c_x/c_y/comp_w
replicated. No collectives (forward only).
"""

import numpy as np

import concourse.bacc as bacc
import concourse.mybir as mybir
import concourse.tile as tile
from concourse.bass import ts
from concourse.bass_utils import run_bass_kernel_spmd
from concourse.masks import make_identity

NCORES = 8
BS, N, DX, DY, M = 256, 128, 32, 10, 2048
BPC = BS // NCORES      # 32 batch items per core
MB = M // 128           # 16 chunks of the component axis
KAUG = DX + 2           # 34: contraction dim with the two folded rows
NBLK = 4                # bn blocks per core
BLKI = BPC // NBLK      # 8 items per block
F_BLK = BLKI * N        # 1024 free elements per (block, m-chunk) tile
MIN_SIGMA = 1e-3
FP32 = mybir.dt.float32
BF16 = mybir.dt.bfloat16
AX = mybir.AxisListType
ALU = mybir.AluOpType
ACTF = mybir.ActivationFunctionType


def _body(tc, inp, cx, cy, cw_d, out_d, scale):
    nc = tc.nc
    from contextlib import ExitStack

    with ExitStack() as ctx:
        const = ctx.enter_context(tc.tile_pool(name="const", bufs=1))
        work = ctx.enter_context(tc.tile_pool(name="work", bufs=2))
        k2p = ctx.enter_context(tc.tile_pool(name="k2p", bufs=3))
        psum = ctx.enter_context(tc.tile_pool(name="psum", bufs=2, space="PSUM"))

        identity = const.tile([128, 128], FP32)
        make_identity(nc, identity)

        # ---- load replicated constants -----------------------------------
        # c_x goes into an augmented (128, 34) layout per chunk: cols 0:32 are
        # c_x, col 32 = 1.0, col 33 = -b2/2.  Transposing the whole tile then
        # yields the augmented C^T rows without any partition-offset writes.
        cx_aug = const.tile([128, MB, KAUG], FP32)
        nc.sync.dma_start(
            out=cx_aug[:, :, 0:DX], in_=cx.rearrange("(t p) d -> p t d", p=128)
        )
        nc.vector.memset(cx_aug[:, :, DX : DX + 1], 1.0)
        cy_all = const.tile([128, MB, DY], FP32)
        nc.sync.dma_start(out=cy_all, in_=cy.rearrange("(t p) d -> p t d", p=128))
        cw_sb = const.tile([128, MB], FP32)
        nc.sync.dma_start(out=cw_sb, in_=cw_d.rearrange("(t p) -> p t", p=128))

        # ---- load batch shard, leaving room for the two augmented cols ----
        A_aug = const.tile([128, BPC, KAUG], FP32)
        nc.sync.dma_start(
            out=A_aug[:, :, 0:DX], in_=inp.rearrange("t p d -> p t d")
        )
        nc.vector.memset(A_aug[:, :, DX + 1 : DX + 2], 1.0)

        # bf16 for both matmul operands: fp32 matmuls stream at 1/4 PE rate.
        CT = const.tile([KAUG, M], BF16)      # augmented C^T (stationary mm1)
        AT = const.tile([KAUG, BPC, N], BF16)  # augmented A^T (moving mm1)
        W_all = const.tile([128, MB, DY + 1], FP32)
        W_bf = const.tile([128, MB, DY + 1], BF16)
        T_sb = const.tile([DY + 1, BPC], FP32)

        # ---- per m-chunk setup: C^T, b2, W -------------------------------
        for mb in range(MB):
            sq = work.tile([128, DX], FP32, tag="sq")
            nc.vector.tensor_mul(sq, cx_aug[:, mb, 0:DX], cx_aug[:, mb, 0:DX])
            nc.vector.tensor_reduce(
                out=cx_aug[:, mb, DX + 1 : DX + 2], in_=sq, axis=AX.X, op=ALU.add
            )
            nc.vector.tensor_scalar_mul(
                cx_aug[:, mb, DX + 1 : DX + 2], cx_aug[:, mb, DX + 1 : DX + 2], -0.5
            )
            trc = psum.tile([KAUG, 128], FP32, tag="g")
            nc.tensor.transpose(trc, cx_aug[:, mb, :], identity)
            nc.vector.tensor_copy(CT[:, ts(mb, 128)], trc)

            sqy = work.tile([128, DY], FP32, tag="sqy")
            nc.vector.tensor_mul(sqy, cy_all[:, mb, :], cy_all[:, mb, :])
            ssum = work.tile([128, 1], FP32, tag="ssum")
            nc.vector.tensor_reduce(out=ssum, in_=sqy, axis=AX.X, op=ALU.add)
            rec = work.tile([128, 1], FP32, tag="rec")
            nc.vector.reciprocal(rec, ssum)
            facr = work.tile([128, 1], FP32, tag="facr")
            nc.vector.tensor_mul(facr, rec, cw_sb[:, mb : mb + 1])
            nc.vector.tensor_scalar(
                out=W_all[:, mb, 0:DY],
                in0=sqy,
                scalar1=facr,
                scalar2=None,
                op0=ALU.mult,
            )
            nc.vector.tensor_copy(W_all[:, mb, DY : DY + 1], cw_sb[:, mb : mb + 1])
        nc.vector.tensor_copy(W_bf, W_all)

        # ---- per item: -a2/2 column, transpose into AT -------------------
        for i in range(BPC):
            sqa = work.tile([128, DX], FP32, tag="sq")
            nc.vector.tensor_mul(sqa, A_aug[:, i, 0:DX], A_aug[:, i, 0:DX])
            nc.vector.tensor_reduce(
                out=A_aug[:, i, DX : DX + 1], in_=sqa, axis=AX.X, op=ALU.add
            )
            nc.vector.tensor_scalar_mul(
                A_aug[:, i, DX : DX + 1], A_aug[:, i, DX : DX + 1], -0.5
            )
            tra = psum.tile([KAUG, 128], FP32, tag="g")
            nc.tensor.transpose(tra, A_aug[:, i, :], identity)
            nc.vector.tensor_copy(AT[:, i, :], tra)

        # ---- main pipeline ------------------------------------------------
        # Software-pipelined: mm2 for step mb-1 is issued after mm1 for step
        # mb, so the PE FIFO never blocks on the ACT output of the current
        # step (otherwise mm1 -> exp -> mm2 serializes each iteration).
        for blk in range(NBLK):
            S = psum.tile([DY + 1, F_BLK], FP32, tag="s")
            k2_prev = None
            for mb in range(MB):
                g = psum.tile([128, F_BLK], FP32, tag="g")
                for q in range(F_BLK // 512):
                    nc.tensor.matmul(
                        g[:, ts(q, 512)],
                        CT[:, ts(mb, 128)],
                        AT[:, blk * BLKI + q * 4 : blk * BLKI + (q + 1) * 4, :],
                        start=True,
                        stop=True,
                    )
                if k2_prev is not None:
                    for q in range(F_BLK // 512):
                        nc.tensor.matmul(
                            S[:, ts(q, 512)],
                            W_bf[:, mb - 1, :],
                            k2_prev[:, ts(q, 512)],
                            start=(mb - 1 == 0),
                            stop=False,
                        )
                K2 = k2p.tile([128, F_BLK], BF16, tag="k2")
                nc.scalar.activation(K2, g, ACTF.Exp, bias=0.0, scale=scale)
                k2_prev = K2
            for q in range(F_BLK // 512):
                nc.tensor.matmul(
                    S[:, ts(q, 512)],
                    W_bf[:, MB - 1, :],
                    k2_prev[:, ts(q, 512)],
                    start=False,
                    stop=True,
                )
            nc.vector.tensor_reduce(
                out=T_sb[:, blk * BLKI : (blk + 1) * BLKI],
                in_=S.rearrange("p (t n) -> p t n", n=N),
                axis=AX.X,
                op=ALU.add,
            )

        # ---- epilogue: probs = T[:, :10] / T[:, 10] -----------------------
        trT = psum.tile([BPC, DY + 1], FP32, tag="g")
        nc.tensor.transpose(trT, T_sb, identity[0 : DY + 1, 0 : DY + 1])
        Tt = const.tile([BPC, DY + 1], FP32)
        nc.vector.tensor_copy(Tt, trT)
        recd = const.tile([BPC, 1], FP32)
        nc.vector.reciprocal(recd, Tt[:, DY : DY + 1])
        outsb = const.tile([BPC, DY], FP32)
        nc.vector.tensor_scalar(
            out=outsb, in0=Tt[:, 0:DY], scalar1=recd, scalar2=None, op0=ALU.mult
        )
        nc.sync.dma_start(out=out_d, in_=outsb)


def build_program(scale):
    nc = bacc.Bacc(
        "TRN2",
        target_bir_lowering=False,
        debug=False,
        enable_asserts=False,
        num_devices=NCORES,
    )
    inp = nc.dram_tensor("inputs", [BPC, N, DX], FP32, kind="ExternalInput").ap()
    cx = nc.dram_tensor("c_x", [M, DX], FP32, kind="ExternalInput").ap()
    cy = nc.dram_tensor("c_y", [M, DY], FP32, kind="ExternalInput").ap()
    cw = nc.dram_tensor("comp_w", [M], FP32, kind="ExternalInput").ap()
    out = nc.dram_tensor("out", [BPC, DY], FP32, kind="ExternalOutput").ap()
    with tile.TileContext(nc) as tc:
        _body(tc, inp, cx, cy, cw, out, scale)
    nc.compile()
    return nc


_PROGRAM_CACHE: dict = {}


def _get_program(scale):
    nc = _PROGRAM_CACHE.get(scale)
    if nc is None:
        nc = build_program(scale)
        _PROGRAM_CACHE[scale] = nc
    return nc


def make_in_maps(inputs, c_x, c_y, comp_w):
    shards = np.ascontiguousarray(inputs.reshape(NCORES, BPC, N, DX))
    return [
        {
            "inputs": shards[i],
            "c_x": np.ascontiguousarray(c_x),
            "c_y": np.ascontiguousarray(c_y),
            "comp_w": np.ascontiguousarray(comp_w),
        }
        for i in range(NCORES)
    ]


def scale_from_sigma(sigma) -> float:
    s = max(float(np.asarray(sigma, dtype=np.float64)), MIN_SIGMA)
    return float(2.0 / (s * s))


def kernel(inputs, sigma, c_x, c_y, comp_w, _run_kwargs=None):
    nc = _get_program(scale_from_sigma(sigma))
    in_maps = make_in_maps(inputs, c_x, c_y, comp_w)
    res = run_bass_kernel_spmd(
        nc, in_maps, core_ids=list(range(NCORES)), **(_run_kwargs or {})
    )
    out = np.concatenate([res.results[i]["out"] for i in range(NCORES)], axis=0)
    return out.astype(np.float32)


# revision 9
# speedup vs baseline: 1.6979x; 1.0065x over previous
"""Trainium2 Bass kernel for nn_BagKQMClassModel.

Computation (per batch item b):
    K[b,n,m]   = exp(-d2/(2 s^2)),  d2 = |A[b,n] - C[m]|^2
    out_w[b,m] = (1/N) sum_n comp_w[m] * K^2
    y_w        = out_w / sum_m out_w
    probs      = y_w @ (y_v^2),  y_v = c_y rows normalized

Key transformations used here:
  * K^2 = exp(-d2/s^2), so only one exp per (b,n,m) element is needed.
  * d2 = a2[bn] + b2[m] - 2 g[m,bn] with g = C @ A^T.  Both -a2/2 and -b2/2
    are folded into the matmul by augmenting the contraction dim (K=34):
        row 32: C^T row = 1,       A^T row = -a2/2
        row 33: C^T row = -b2/2,   A^T row = 1
    so one matmul emits g_full with exp argument = (2/s^2) * g_full and the
    activation needs no bias at all.
  * probs = T[:, :10] / T[:, 10] where T[b,:] = sum_n sum_m K2[m,bn]*W[m,:],
    W[m, :10] = comp_w[m] * c_y[m]^2 / |c_y[m]|^2,  W[m, 10] = comp_w[m].
    The 1/N bag weight and normalization cancel.
  * Layout (m on partitions, b*n on free dim): matmul2 contracts m on the PE
    with W as the stationary operand; the final n-reduction happens on the
    tiny (11, bn) result via a segmented DVE reduce.

Sharding: batch 256 -> 32 items per core across 8 cores; c_x/c_y/comp_w
replicated. No collectives (forward only).
"""

import numpy as np

import concourse.bacc as bacc
import concourse.mybir as mybir
import concourse.tile as tile
from concourse.bass import ts
from concourse.bass_utils import run_bass_kernel_spmd
from concourse.masks import make_identity

NCORES = 8
BS, N, DX, DY, M = 256, 128, 32, 10, 2048
BPC = BS // NCORES      # 32 batch items per core
MB = M // 128           # 16 chunks of the component axis
KAUG = DX + 2           # 34: contraction dim with the two folded rows
NBLK = 4                # bn blocks per core
BLKI = BPC // NBLK      # 8 items per block
F_BLK = BLKI * N        # 1024 free elements per (block, m-chunk) tile
MIN_SIGMA = 1e-3
FP32 = mybir.dt.float32
BF16 = mybir.dt.bfloat16
AX = mybir.AxisListType
ALU = mybir.AluOpType
ACTF = mybir.ActivationFunctionType


def _body(tc, inp, cx, cy, cw_d, out_d, scale):
    nc = tc.nc
    from contextlib import ExitStack

    with ExitStack() as ctx:
        const = ctx.enter_context(tc.tile_pool(name="const", bufs=1))
        work = ctx.enter_context(tc.tile_pool(name="work", bufs=2))
        k2p = ctx.enter_context(tc.tile_pool(name="k2p", bufs=3))
        psum = ctx.enter_context(tc.tile_pool(name="psum", bufs=2, space="PSUM"))

        identity = const.tile([128, 128], FP32)
        make_identity(nc, identity)

        # ---- load replicated constants -----------------------------------
        # c_x goes into an augmented (128, 34) layout per chunk: cols 0:32 are
        # c_x, col 32 = 1.0, col 33 = -b2/2.  Transposing the whole tile then
        # yields the augmented C^T rows without any partition-offset writes.
        cx_aug = const.tile([128, MB, KAUG], FP32)
        nc.sync.dma_start(
            out=cx_aug[:, :, 0:DX], in_=cx.rearrange("(t p) d -> p t d", p=128)
        )
        nc.vector.memset(cx_aug[:, :, DX : DX + 1], 1.0)
        cy_all = const.tile([128, MB, DY], FP32)
        nc.sync.dma_start(out=cy_all, in_=cy.rearrange("(t p) d -> p t d", p=128))
        cw_sb = const.tile([128, MB], FP32)
        nc.sync.dma_start(out=cw_sb, in_=cw_d.rearrange("(t p) -> p t", p=128))

        # ---- load batch shard, leaving room for the two augmented cols ----
        A_aug = const.tile([128, BPC, KAUG], FP32)
        nc.sync.dma_start(
            out=A_aug[:, :, 0:DX], in_=inp.rearrange("t p d -> p t d")
        )
        nc.vector.memset(A_aug[:, :, DX + 1 : DX + 2], 1.0)

        # bf16 for both matmul operands: fp32 matmuls stream at 1/4 PE rate.
        CT = const.tile([KAUG, M], BF16)      # augmented C^T (stationary mm1)
        AT = const.tile([KAUG, BPC, N], BF16)  # augmented A^T (moving mm1)
        W_all = const.tile([128, MB, DY + 1], FP32)
        W_bf = const.tile([128, MB, DY + 1], BF16)
        T_sb = const.tile([DY + 1, BPC], FP32)

        # ---- per m-chunk setup: C^T, b2, W -------------------------------
        for mb in range(MB):
            sq = work.tile([128, DX], FP32, tag="sq")
            nc.vector.tensor_mul(sq, cx_aug[:, mb, 0:DX], cx_aug[:, mb, 0:DX])
            nc.vector.tensor_reduce(
                out=cx_aug[:, mb, DX + 1 : DX + 2], in_=sq, axis=AX.X, op=ALU.add
            )
            nc.vector.tensor_scalar_mul(
                cx_aug[:, mb, DX + 1 : DX + 2], cx_aug[:, mb, DX + 1 : DX + 2], -0.5
            )
            trc = psum.tile([KAUG, 128], FP32, tag="g", bufs=3)
            nc.tensor.transpose(trc, cx_aug[:, mb, :], identity)
            nc.vector.tensor_copy(CT[:, ts(mb, 128)], trc)

            sqy = work.tile([128, DY], FP32, tag="sqy")
            nc.vector.tensor_mul(sqy, cy_all[:, mb, :], cy_all[:, mb, :])
            ssum = work.tile([128, 1], FP32, tag="ssum")
            nc.vector.tensor_reduce(out=ssum, in_=sqy, axis=AX.X, op=ALU.add)
            rec = work.tile([128, 1], FP32, tag="rec")
            nc.vector.reciprocal(rec, ssum)
            facr = work.tile([128, 1], FP32, tag="facr")
            nc.vector.tensor_mul(facr, rec, cw_sb[:, mb : mb + 1])
            nc.vector.tensor_scalar(
                out=W_all[:, mb, 0:DY],
                in0=sqy,
                scalar1=facr,
                scalar2=None,
                op0=ALU.mult,
            )
            nc.vector.tensor_copy(W_all[:, mb, DY : DY + 1], cw_sb[:, mb : mb + 1])
        nc.vector.tensor_copy(W_bf, W_all)

        # ---- per item: -a2/2 column, transpose into AT -------------------
        for i in range(BPC):
            sqa = work.tile([128, DX], FP32, tag="sq")
            nc.vector.tensor_mul(sqa, A_aug[:, i, 0:DX], A_aug[:, i, 0:DX])
            nc.vector.tensor_reduce(
                out=A_aug[:, i, DX : DX + 1], in_=sqa, axis=AX.X, op=ALU.add
            )
            nc.vector.tensor_scalar_mul(
                A_aug[:, i, DX : DX + 1], A_aug[:, i, DX : DX + 1], -0.5
            )
            tra = psum.tile([KAUG, 128], FP32, tag="g", bufs=3)
            nc.tensor.transpose(tra, A_aug[:, i, :], identity)
            nc.vector.tensor_copy(AT[:, i, :], tra)

        # ---- main pipeline ------------------------------------------------
        # Software-pipelined: mm2 for step mb-1 is issued after mm1 for step
        # mb, so the PE FIFO never blocks on the ACT output of the current
        # step (otherwise mm1 -> exp -> mm2 serializes each iteration).
        for blk in range(NBLK):
            S = psum.tile([DY + 1, F_BLK], FP32, tag="s", bufs=1)
            k2_prev = None
            for mb in range(MB):
                g = psum.tile([128, F_BLK], FP32, tag="g", bufs=3)
                for q in range(F_BLK // 512):
                    nc.tensor.matmul(
                        g[:, ts(q, 512)],
                        CT[:, ts(mb, 128)],
                        AT[:, blk * BLKI + q * 4 : blk * BLKI + (q + 1) * 4, :],
                        start=True,
                        stop=True,
                    )
                if k2_prev is not None:
                    for q in range(F_BLK // 512):
                        nc.tensor.matmul(
                            S[:, ts(q, 512)],
                            W_bf[:, mb - 1, :],
                            k2_prev[:, ts(q, 512)],
                            start=(mb - 1 == 0),
                            stop=False,
                        )
                K2 = k2p.tile([128, F_BLK], BF16, tag="k2")
                nc.scalar.activation(K2, g, ACTF.Exp, bias=0.0, scale=scale)
                k2_prev = K2
            for q in range(F_BLK // 512):
                nc.tensor.matmul(
                    S[:, ts(q, 512)],
                    W_bf[:, MB - 1, :],
                    k2_prev[:, ts(q, 512)],
                    start=False,
                    stop=True,
                )
            nc.vector.tensor_reduce(
                out=T_sb[:, blk * BLKI : (blk + 1) * BLKI],
                in_=S.rearrange("p (t n) -> p t n", n=N),
                axis=AX.X,
                op=ALU.add,
            )

        # ---- epilogue: probs = T[:, :10] / T[:, 10] -----------------------
        trT = psum.tile([BPC, DY + 1], FP32, tag="g", bufs=3)
        nc.tensor.transpose(trT, T_sb, identity[0 : DY + 1, 0 : DY + 1])
        Tt = const.tile([BPC, DY + 1], FP32)
        nc.vector.tensor_copy(Tt, trT)
        recd = const.tile([BPC, 1], FP32)
        nc.vector.reciprocal(recd, Tt[:, DY : DY + 1])
        outsb = const.tile([BPC, DY], FP32)
        nc.vector.tensor_scalar(
            out=outsb, in0=Tt[:, 0:DY], scalar1=recd, scalar2=None, op0=ALU.mult
        )
        nc.sync.dma_start(out=out_d, in_=outsb)


def build_program(scale):
    nc = bacc.Bacc(
        "TRN2",
        target_bir_lowering=False,
        debug=False,
        enable_asserts=False,
        num_devices=NCORES,
    )
    inp = nc.dram_tensor("inputs", [BPC, N, DX], FP32, kind="ExternalInput").ap()
    cx = nc.dram_tensor("c_x", [M, DX], FP32, kind="ExternalInput").ap()
    cy = nc.dram_tensor("c_y", [M, DY], FP32, kind="ExternalInput").ap()
    cw = nc.dram_tensor("comp_w", [M], FP32, kind="ExternalInput").ap()
    out = nc.dram_tensor("out", [BPC, DY], FP32, kind="ExternalOutput").ap()
    with tile.TileContext(nc) as tc:
        _body(tc, inp, cx, cy, cw, out, scale)
    nc.compile()
    return nc


_PROGRAM_CACHE: dict = {}


def _get_program(scale):
    nc = _PROGRAM_CACHE.get(scale)
    if nc is None:
        nc = build_program(scale)
        _PROGRAM_CACHE[scale] = nc
    return nc


def make_in_maps(inputs, c_x, c_y, comp_w):
    shards = np.ascontiguousarray(inputs.reshape(NCORES, BPC, N, DX))
    return [
        {
            "inputs": shards[i],
            "c_x": np.ascontiguousarray(c_x),
            "c_y": np.ascontiguousarray(c_y),
            "comp_w": np.ascontiguousarray(comp_w),
        }
        for i in range(NCORES)
    ]


def scale_from_sigma(sigma) -> float:
    s = max(float(np.asarray(sigma, dtype=np.float64)), MIN_SIGMA)
    return float(2.0 / (s * s))


def kernel(inputs, sigma, c_x, c_y, comp_w, _run_kwargs=None):
    nc = _get_program(scale_from_sigma(sigma))
    in_maps = make_in_maps(inputs, c_x, c_y, comp_w)
    res = run_bass_kernel_spmd(
        nc, in_maps, core_ids=list(range(NCORES)), **(_run_kwargs or {})
    )
    out = np.concatenate([res.results[i]["out"] for i in range(NCORES)], axis=0)
    return out.astype(np.float32)


# revision 10
# speedup vs baseline: 1.7196x; 1.0128x over previous
"""Trainium2 Bass kernel for nn_BagKQMClassModel.

Computation (per batch item b):
    K[b,n,m]   = exp(-d2/(2 s^2)),  d2 = |A[b,n] - C[m]|^2
    out_w[b,m] = (1/N) sum_n comp_w[m] * K^2
    y_w        = out_w / sum_m out_w
    probs      = y_w @ (y_v^2),  y_v = c_y rows normalized

Key transformations used here:
  * K^2 = exp(-d2/s^2), so only one exp per (b,n,m) element is needed.
  * d2 = a2[bn] + b2[m] - 2 g[m,bn] with g = C @ A^T.  Both -a2/2 and -b2/2
    are folded into the matmul by augmenting the contraction dim (K=34):
        row 32: C^T row = 1,       A^T row = -a2/2
        row 33: C^T row = -b2/2,   A^T row = 1
    so one matmul emits g_full with exp argument = (2/s^2) * g_full and the
    activation needs no bias at all.
  * probs = T[:, :10] / T[:, 10] where T[b,:] = sum_n sum_m K2[m,bn]*W[m,:],
    W[m, :10] = comp_w[m] * c_y[m]^2 / |c_y[m]|^2,  W[m, 10] = comp_w[m].
    The 1/N bag weight and normalization cancel.
  * Layout (m on partitions, b*n on free dim): matmul2 contracts m on the PE
    with W as the stationary operand; the final n-reduction happens on the
    tiny (11, bn) result via a segmented DVE reduce.

Sharding: batch 256 -> 32 items per core across 8 cores; c_x/c_y/comp_w
replicated. No collectives (forward only).
"""

import numpy as np

import concourse.bacc as bacc
import concourse.mybir as mybir
import concourse.tile as tile
from concourse.bass import ts
from concourse.bass_utils import run_bass_kernel_spmd
from concourse.masks import make_identity

NCORES = 8
BS, N, DX, DY, M = 256, 128, 32, 10, 2048
BPC = BS // NCORES      # 32 batch items per core
MB = M // 128           # 16 chunks of the component axis
KAUG = DX + 2           # 34: contraction dim with the two folded rows
NBLK = 4                # bn blocks per core
BLKI = BPC // NBLK      # 8 items per block
F_BLK = BLKI * N        # 1024 free elements per (block, m-chunk) tile
MIN_SIGMA = 1e-3
FP32 = mybir.dt.float32
BF16 = mybir.dt.bfloat16
AX = mybir.AxisListType
ALU = mybir.AluOpType
ACTF = mybir.ActivationFunctionType


def _body(tc, inp, cx, cy, cw_d, out_d, scale):
    nc = tc.nc
    from contextlib import ExitStack

    with ExitStack() as ctx:
        const = ctx.enter_context(tc.tile_pool(name="const", bufs=1))
        work = ctx.enter_context(tc.tile_pool(name="work", bufs=2))
        k2p = ctx.enter_context(tc.tile_pool(name="k2p", bufs=3))
        psum = ctx.enter_context(tc.tile_pool(name="psum", bufs=2, space="PSUM"))

        identity = const.tile([128, 128], FP32)
        make_identity(nc, identity)

        # ---- HAM warm-up ---------------------------------------------------
        # The PE clock sits at 1.2 GHz until the activity monitor sees a full
        # ~3.4us window of sustained matmul work; the steady-state here never
        # has >2.5us of contiguous PE time, so without this burst the whole
        # kernel runs at half clock.  ~12 back-to-back dummy matmuls (chained
        # by WAW on one psum tile) run during the input DMAs and trip the
        # monitor once; the later gaps are far too short to re-throttle.
        warm_src = const.tile([128, 512], BF16)
        nc.vector.memset(warm_src, 0.0)
        warm_ps = psum.tile([128, 512], FP32, tag="s", bufs=1)
        warm_w = warm_src[:, 0:128]
        for _ in range(12):
            nc.tensor.matmul(warm_ps, warm_w, warm_src, start=True, stop=True)

        # ---- load replicated constants -----------------------------------
        # c_x goes into an augmented (128, 34) layout per chunk: cols 0:32 are
        # c_x, col 32 = 1.0, col 33 = -b2/2.  Transposing the whole tile then
        # yields the augmented C^T rows without any partition-offset writes.
        cx_aug = const.tile([128, MB, KAUG], FP32)
        nc.sync.dma_start(
            out=cx_aug[:, :, 0:DX], in_=cx.rearrange("(t p) d -> p t d", p=128)
        )
        nc.vector.memset(cx_aug[:, :, DX : DX + 1], 1.0)
        cy_all = const.tile([128, MB, DY], FP32)
        nc.sync.dma_start(out=cy_all, in_=cy.rearrange("(t p) d -> p t d", p=128))
        cw_sb = const.tile([128, MB], FP32)
        nc.sync.dma_start(out=cw_sb, in_=cw_d.rearrange("(t p) -> p t", p=128))

        # ---- load batch shard, leaving room for the two augmented cols ----
        A_aug = const.tile([128, BPC, KAUG], FP32)
        nc.sync.dma_start(
            out=A_aug[:, :, 0:DX], in_=inp.rearrange("t p d -> p t d")
        )
        nc.vector.memset(A_aug[:, :, DX + 1 : DX + 2], 1.0)

        # bf16 for both matmul operands: fp32 matmuls stream at 1/4 PE rate.
        CT = const.tile([KAUG, M], BF16)      # augmented C^T (stationary mm1)
        AT = const.tile([KAUG, BPC, N], BF16)  # augmented A^T (moving mm1)
        W_all = const.tile([128, MB, DY + 1], FP32)
        W_bf = const.tile([128, MB, DY + 1], BF16)
        T_sb = const.tile([DY + 1, BPC], FP32)

        # ---- per m-chunk setup: C^T, b2, W -------------------------------
        for mb in range(MB):
            sq = work.tile([128, DX], FP32, tag="sq")
            nc.vector.tensor_mul(sq, cx_aug[:, mb, 0:DX], cx_aug[:, mb, 0:DX])
            nc.vector.tensor_reduce(
                out=cx_aug[:, mb, DX + 1 : DX + 2], in_=sq, axis=AX.X, op=ALU.add
            )
            nc.vector.tensor_scalar_mul(
                cx_aug[:, mb, DX + 1 : DX + 2], cx_aug[:, mb, DX + 1 : DX + 2], -0.5
            )
            trc = psum.tile([KAUG, 128], FP32, tag="g", bufs=3)
            nc.tensor.transpose(trc, cx_aug[:, mb, :], identity)
            nc.vector.tensor_copy(CT[:, ts(mb, 128)], trc)

            sqy = work.tile([128, DY], FP32, tag="sqy")
            nc.vector.tensor_mul(sqy, cy_all[:, mb, :], cy_all[:, mb, :])
            ssum = work.tile([128, 1], FP32, tag="ssum")
            nc.vector.tensor_reduce(out=ssum, in_=sqy, axis=AX.X, op=ALU.add)
            rec = work.tile([128, 1], FP32, tag="rec")
            nc.vector.reciprocal(rec, ssum)
            facr = work.tile([128, 1], FP32, tag="facr")
            nc.vector.tensor_mul(facr, rec, cw_sb[:, mb : mb + 1])
            nc.vector.tensor_scalar(
                out=W_all[:, mb, 0:DY],
                in0=sqy,
                scalar1=facr,
                scalar2=None,
                op0=ALU.mult,
            )
            nc.vector.tensor_copy(W_all[:, mb, DY : DY + 1], cw_sb[:, mb : mb + 1])
        nc.vector.tensor_copy(W_bf, W_all)

        # ---- per item: -a2/2 column, transpose into AT -------------------
        for i in range(BPC):
            sqa = work.tile([128, DX], FP32, tag="sq")
            nc.vector.tensor_mul(sqa, A_aug[:, i, 0:DX], A_aug[:, i, 0:DX])
            nc.vector.tensor_reduce(
                out=A_aug[:, i, DX : DX + 1], in_=sqa, axis=AX.X, op=ALU.add
            )
            nc.vector.tensor_scalar_mul(
                A_aug[:, i, DX : DX + 1], A_aug[:, i, DX : DX + 1], -0.5
            )
            tra = psum.tile([KAUG, 128], FP32, tag="g", bufs=3)
            nc.tensor.transpose(tra, A_aug[:, i, :], identity)
            nc.vector.tensor_copy(AT[:, i, :], tra)

        # ---- main pipeline ------------------------------------------------
        # Software-pipelined: mm2 for step mb-1 is issued after mm1 for step
        # mb, so the PE FIFO never blocks on the ACT output of the current
        # step (otherwise mm1 -> exp -> mm2 serializes each iteration).
        for blk in range(NBLK):
            S = psum.tile([DY + 1, F_BLK], FP32, tag="s", bufs=1)
            k2_prev = None
            for mb in range(MB):
                g = psum.tile([128, F_BLK], FP32, tag="g", bufs=3)
                for q in range(F_BLK // 512):
                    nc.tensor.matmul(
                        g[:, ts(q, 512)],
                        CT[:, ts(mb, 128)],
                        AT[:, blk * BLKI + q * 4 : blk * BLKI + (q + 1) * 4, :],
                        start=True,
                        stop=True,
                    )
                if k2_prev is not None:
                    for q in range(F_BLK // 512):
                        nc.tensor.matmul(
                            S[:, ts(q, 512)],
                            W_bf[:, mb - 1, :],
                            k2_prev[:, ts(q, 512)],
                            start=(mb - 1 == 0),
                            stop=False,
                        )
                K2 = k2p.tile([128, F_BLK], BF16, tag="k2")
                nc.scalar.activation(K2, g, ACTF.Exp, bias=0.0, scale=scale)
                k2_prev = K2
            for q in range(F_BLK // 512):
                nc.tensor.matmul(
                    S[:, ts(q, 512)],
                    W_bf[:, MB - 1, :],
                    k2_prev[:, ts(q, 512)],
                    start=False,
                    stop=True,
                )
            nc.vector.tensor_reduce(
                out=T_sb[:, blk * BLKI : (blk + 1) * BLKI],
                in_=S.rearrange("p (t n) -> p t n", n=N),
                axis=AX.X,
                op=ALU.add,
            )

        # ---- epilogue: probs = T[:, :10] / T[:, 10] -----------------------
        trT = psum.tile([BPC, DY + 1], FP32, tag="g", bufs=3)
        nc.tensor.transpose(trT, T_sb, identity[0 : DY + 1, 0 : DY + 1])
        Tt = const.tile([BPC, DY + 1], FP32)
        nc.vector.tensor_copy(Tt, trT)
        recd = const.tile([BPC, 1], FP32)
        nc.vector.reciprocal(recd, Tt[:, DY : DY + 1])
        outsb = const.tile([BPC, DY], FP32)
        nc.vector.tensor_scalar(
            out=outsb, in0=Tt[:, 0:DY], scalar1=recd, scalar2=None, op0=ALU.mult
        )
        nc.sync.dma_start(out=out_d, in_=outsb)


def build_program(scale):
    nc = bacc.Bacc(
        "TRN2",
        target_bir_lowering=False,
        debug=False,
        enable_asserts=False,
        num_devices=NCORES,
    )
    inp = nc.dram_tensor("inputs", [BPC, N, DX], FP32, kind="ExternalInput").ap()
    cx = nc.dram_tensor("c_x", [M, DX], FP32, kind="ExternalInput").ap()
    cy = nc.dram_tensor("c_y", [M, DY], FP32, kind="ExternalInput").ap()
    cw = nc.dram_tensor("comp_w", [M], FP32, kind="ExternalInput").ap()
    out = nc.dram_tensor("out", [BPC, DY], FP32, kind="ExternalOutput").ap()
    with tile.TileContext(nc) as tc:
        _body(tc, inp, cx, cy, cw, out, scale)
    nc.compile()
    return nc


_PROGRAM_CACHE: dict = {}


def _get_program(scale):
    nc = _PROGRAM_CACHE.get(scale)
    if nc is None:
        nc = build_program(scale)
        _PROGRAM_CACHE[scale] = nc
    return nc


def make_in_maps(inputs, c_x, c_y, comp_w):
    shards = np.ascontiguousarray(inputs.reshape(NCORES, BPC, N, DX))
    return [
        {
            "inputs": shards[i],
            "c_x": np.ascontiguousarray(c_x),
            "c_y": np.ascontiguousarray(c_y),
            "comp_w": np.ascontiguousarray(comp_w),
        }
        for i in range(NCORES)
    ]


def scale_from_sigma(sigma) -> float:
    s = max(float(np.asarray(sigma, dtype=np.float64)), MIN_SIGMA)
    return float(2.0 / (s * s))


def kernel(inputs, sigma, c_x, c_y, comp_w, _run_kwargs=None):
    nc = _get_program(scale_from_sigma(sigma))
    in_maps = make_in_maps(inputs, c_x, c_y, comp_w)
    res = run_bass_kernel_spmd(
        nc, in_maps, core_ids=list(range(NCORES)), **(_run_kwargs or {})
    )
    out = np.concatenate([res.results[i]["out"] for i in range(NCORES)], axis=0)
    return out.astype(np.float32)


# revision 12
# speedup vs baseline: 1.8069x; 1.0508x over previous
"""Trainium2 Bass kernel for nn_BagKQMClassModel.

Computation (per batch item b):
    K[b,n,m]   = exp(-d2/(2 s^2)),  d2 = |A[b,n] - C[m]|^2
    out_w[b,m] = (1/N) sum_n comp_w[m] * K^2
    y_w        = out_w / sum_m out_w
    probs      = y_w @ (y_v^2),  y_v = c_y rows normalized

Key transformations used here:
  * K^2 = exp(-d2/s^2), so only one exp per (b,n,m) element is needed.
  * d2 = a2[bn] + b2[m] - 2 g[m,bn] with g = C @ A^T.  Both -a2/2 and -b2/2
    are folded into the matmul by augmenting the contraction dim (K=34):
        row 32: C^T row = 1,       A^T row = -a2/2
        row 33: C^T row = -b2/2,   A^T row = 1
    so one matmul emits g_full with exp argument = (2/s^2) * g_full and the
    activation needs no bias at all.
  * probs = T[:, :10] / T[:, 10] where T[b,:] = sum_n sum_m K2[m,bn]*W[m,:],
    W[m, :10] = comp_w[m] * c_y[m]^2 / |c_y[m]|^2,  W[m, 10] = comp_w[m].
    The 1/N bag weight and normalization cancel.
  * Layout (m on partitions, b*n on free dim): matmul2 contracts m on the PE
    with W as the stationary operand; the final n-reduction happens on the
    tiny (11, bn) result via a segmented DVE reduce.

Sharding: batch 256 -> 32 items per core across 8 cores; c_x/c_y/comp_w
replicated. No collectives (forward only).
"""

import numpy as np

import concourse.bacc as bacc
import concourse.mybir as mybir
import concourse.tile as tile
from concourse.bass import ts
from concourse.bass_utils import run_bass_kernel_spmd
from concourse.masks import make_identity

NCORES = 8
BS, N, DX, DY, M = 256, 128, 32, 10, 2048
BPC = BS // NCORES      # 32 batch items per core
MB = M // 128           # 16 chunks of the component axis
KAUG = DX + 2           # 34: contraction dim with the two folded rows
NBLK = 4                # bn blocks per core
BLKI = BPC // NBLK      # 8 items per block
F_BLK = BLKI * N        # 1024 free elements per (block, m-chunk) tile
MIN_SIGMA = 1e-3
FP32 = mybir.dt.float32
BF16 = mybir.dt.bfloat16
AX = mybir.AxisListType
ALU = mybir.AluOpType
ACTF = mybir.ActivationFunctionType


def _body(tc, inp, cx, cy, cw_d, out_d, scale):
    nc = tc.nc
    from contextlib import ExitStack

    with ExitStack() as ctx:
        const = ctx.enter_context(tc.tile_pool(name="const", bufs=1))
        work = ctx.enter_context(tc.tile_pool(name="work", bufs=2))
        k2p = ctx.enter_context(tc.tile_pool(name="k2p", bufs=3))
        psum = ctx.enter_context(tc.tile_pool(name="psum", bufs=2, space="PSUM"))

        identity = const.tile([128, 128], FP32)
        make_identity(nc, identity)

        # ---- HAM warm-up ---------------------------------------------------
        # The PE clock sits at 1.2 GHz until the activity monitor sees a full
        # ~3.4us window of sustained matmul work; the steady-state here never
        # has >2.5us of contiguous PE time, so without this burst the whole
        # kernel runs at half clock.  ~12 back-to-back dummy matmuls (chained
        # by WAW on one psum tile) run during the input DMAs and trip the
        # monitor once; the later gaps are far too short to re-throttle.
        warm_src = const.tile([128, 512], BF16)
        nc.vector.memset(warm_src, 0.0)
        warm_ps = psum.tile([128, 512], FP32, tag="s", bufs=1)
        warm_w = warm_src[:, 0:128]
        for _ in range(16):
            nc.tensor.matmul(warm_ps, warm_w, warm_src, start=True, stop=True)

        # ---- load replicated constants -----------------------------------
        # c_x goes into an augmented (128, 34) layout per chunk: cols 0:32 are
        # c_x, col 32 = 1.0, col 33 = -b2/2.  Transposing the whole tile then
        # yields the augmented C^T rows without any partition-offset writes.
        cx_aug = const.tile([128, MB, KAUG], FP32)
        nc.sync.dma_start(
            out=cx_aug[:, :, 0:DX], in_=cx.rearrange("(t p) d -> p t d", p=128)
        )
        nc.vector.memset(cx_aug[:, :, DX : DX + 1], 1.0)
        cy_all = const.tile([128, MB, DY], FP32)
        nc.sync.dma_start(out=cy_all, in_=cy.rearrange("(t p) d -> p t d", p=128))
        cw_sb = const.tile([128, MB], FP32)
        nc.sync.dma_start(out=cw_sb, in_=cw_d.rearrange("(t p) -> p t", p=128))

        # ---- load batch shard, leaving room for the two augmented cols ----
        A_aug = const.tile([128, BPC, KAUG], FP32)
        nc.sync.dma_start(
            out=A_aug[:, :, 0:DX], in_=inp.rearrange("t p d -> p t d")
        )
        nc.vector.memset(A_aug[:, :, DX + 1 : DX + 2], 1.0)

        # bf16 for both matmul operands: fp32 matmuls stream at 1/4 PE rate.
        CT = const.tile([KAUG, M], BF16)      # augmented C^T (stationary mm1)
        AT = const.tile([KAUG, BPC, N], BF16)  # augmented A^T (moving mm1)
        W_all = const.tile([128, MB, DY + 1], FP32)
        W_bf = const.tile([128, MB, DY + 1], BF16)
        T_sb = const.tile([DY + 1, BPC], FP32)

        # ---- m-chunk setup: b2 column of cx_aug, then W (all batched) -----
        sqx = work.tile([128, MB, DX], FP32, tag="sqx")
        nc.vector.tensor_mul(sqx, cx_aug[:, :, 0:DX], cx_aug[:, :, 0:DX])
        nc.vector.tensor_reduce(
            out=cx_aug[:, :, DX + 1 : DX + 2], in_=sqx, axis=AX.X, op=ALU.add
        )
        nc.vector.tensor_scalar_mul(
            cx_aug[:, :, DX + 1 : DX + 2], cx_aug[:, :, DX + 1 : DX + 2], -0.5
        )

        sqy = work.tile([128, MB, DY], FP32, tag="sqy")
        nc.vector.tensor_mul(sqy, cy_all, cy_all)
        ssum = work.tile([128, MB], FP32, tag="ssum")
        nc.vector.tensor_reduce(out=ssum, in_=sqy, axis=AX.X, op=ALU.add)
        rec = work.tile([128, MB], FP32, tag="rec")
        nc.vector.reciprocal(rec, ssum)
        facr = work.tile([128, MB], FP32, tag="facr")
        nc.vector.tensor_mul(facr, rec, cw_sb)
        for mb in range(MB):
            nc.vector.tensor_scalar(
                out=W_all[:, mb, 0:DY],
                in0=sqy[:, mb, :],
                scalar1=facr[:, mb : mb + 1],
                scalar2=None,
                op0=ALU.mult,
            )
        nc.vector.tensor_copy(
            W_all[:, :, DY : DY + 1], cw_sb.rearrange("p (t one) -> p t one", one=1)
        )
        nc.vector.tensor_copy(W_bf, W_all)

        # ---- batched -a2/2 column for all items ---------------------------
        sqa = work.tile([128, BPC, DX], FP32, tag="sqa")
        nc.vector.tensor_mul(sqa, A_aug[:, :, 0:DX], A_aug[:, :, 0:DX])
        nc.vector.tensor_reduce(
            out=A_aug[:, :, DX : DX + 1], in_=sqa, axis=AX.X, op=ALU.add
        )
        nc.vector.tensor_scalar_mul(
            A_aug[:, :, DX : DX + 1], A_aug[:, :, DX : DX + 1], -0.5
        )

        # ---- transposes (PE), psum->sbuf casts on the idle ACT engine -----
        for mb in range(MB):
            trc = psum.tile([KAUG, 128], FP32, tag="g", bufs=3)
            nc.tensor.transpose(trc, cx_aug[:, mb, :], identity)
            nc.scalar.copy(CT[:, ts(mb, 128)], trc)
        for i in range(BPC):
            tra = psum.tile([KAUG, 128], FP32, tag="g", bufs=3)
            nc.tensor.transpose(tra, A_aug[:, i, :], identity)
            nc.scalar.copy(AT[:, i, :], tra)

        # insurance warm-up: if the HAM re-throttled during the DVE-heavy
        # setup, this re-warms the PE right before the steady-state loop.
        for _ in range(10):
            nc.tensor.matmul(warm_ps, CT[:, 0:128], AT[:, 0:4, :], start=True, stop=True)

        # ---- main pipeline ------------------------------------------------
        # Software-pipelined: mm2 for step mb-1 is issued after mm1 for step
        # mb, so the PE FIFO never blocks on the ACT output of the current
        # step (otherwise mm1 -> exp -> mm2 serializes each iteration).
        for blk in range(NBLK):
            S = psum.tile([DY + 1, F_BLK], FP32, tag="s", bufs=1)
            k2_prev = None
            for mb in range(MB):
                g = psum.tile([128, F_BLK], FP32, tag="g", bufs=3)
                for q in range(F_BLK // 512):
                    nc.tensor.matmul(
                        g[:, ts(q, 512)],
                        CT[:, ts(mb, 128)],
                        AT[:, blk * BLKI + q * 4 : blk * BLKI + (q + 1) * 4, :],
                        start=True,
                        stop=True,
                    )
                if k2_prev is not None:
                    for q in range(F_BLK // 512):
                        nc.tensor.matmul(
                            S[:, ts(q, 512)],
                            W_bf[:, mb - 1, :],
                            k2_prev[:, ts(q, 512)],
                            start=(mb - 1 == 0),
                            stop=False,
                        )
                K2 = k2p.tile([128, F_BLK], BF16, tag="k2")
                nc.scalar.activation(K2, g, ACTF.Exp, bias=0.0, scale=scale)
                k2_prev = K2
            for q in range(F_BLK // 512):
                nc.tensor.matmul(
                    S[:, ts(q, 512)],
                    W_bf[:, MB - 1, :],
                    k2_prev[:, ts(q, 512)],
                    start=False,
                    stop=True,
                )
            nc.vector.tensor_reduce(
                out=T_sb[:, blk * BLKI : (blk + 1) * BLKI],
                in_=S.rearrange("p (t n) -> p t n", n=N),
                axis=AX.X,
                op=ALU.add,
            )

        # ---- epilogue: probs = T[:, :10] / T[:, 10] -----------------------
        trT = psum.tile([BPC, DY + 1], FP32, tag="g", bufs=3)
        nc.tensor.transpose(trT, T_sb, identity[0 : DY + 1, 0 : DY + 1])
        Tt = const.tile([BPC, DY + 1], FP32)
        nc.vector.tensor_copy(Tt, trT)
        recd = const.tile([BPC, 1], FP32)
        nc.vector.reciprocal(recd, Tt[:, DY : DY + 1])
        outsb = const.tile([BPC, DY], FP32)
        nc.vector.tensor_scalar(
            out=outsb, in0=Tt[:, 0:DY], scalar1=recd, scalar2=None, op0=ALU.mult
        )
        nc.sync.dma_start(out=out_d, in_=outsb)


def build_program(scale):
    nc = bacc.Bacc(
        "TRN2",
        target_bir_lowering=False,
        debug=False,
        enable_asserts=False,
        num_devices=NCORES,
    )
    inp = nc.dram_tensor("inputs", [BPC, N, DX], FP32, kind="ExternalInput").ap()
    cx = nc.dram_tensor("c_x", [M, DX], FP32, kind="ExternalInput").ap()
    cy = nc.dram_tensor("c_y", [M, DY], FP32, kind="ExternalInput").ap()
    cw = nc.dram_tensor("comp_w", [M], FP32, kind="ExternalInput").ap()
    out = nc.dram_tensor("out", [BPC, DY], FP32, kind="ExternalOutput").ap()
    with tile.TileContext(nc) as tc:
        _body(tc, inp, cx, cy, cw, out, scale)
    nc.compile()
    return nc


_PROGRAM_CACHE: dict = {}


def _get_program(scale):
    nc = _PROGRAM_CACHE.get(scale)
    if nc is None:
        nc = build_program(scale)
        _PROGRAM_CACHE[scale] = nc
    return nc


def make_in_maps(inputs, c_x, c_y, comp_w):
    shards = np.ascontiguousarray(inputs.reshape(NCORES, BPC, N, DX))
    return [
        {
            "inputs": shards[i],
            "c_x": np.ascontiguousarray(c_x),
            "c_y": np.ascontiguousarray(c_y),
            "comp_w": np.ascontiguousarray(comp_w),
        }
        for i in range(NCORES)
    ]


def scale_from_sigma(sigma) -> float:
    s = max(float(np.asarray(sigma, dtype=np.float64)), MIN_SIGMA)
    return float(2.0 / (s * s))


def kernel(inputs, sigma, c_x, c_y, comp_w, _run_kwargs=None):
    nc = _get_program(scale_from_sigma(sigma))
    in_maps = make_in_maps(inputs, c_x, c_y, comp_w)
    res = run_bass_kernel_spmd(
        nc, in_maps, core_ids=list(range(NCORES)), **(_run_kwargs or {})
    )
    out = np.concatenate([res.results[i]["out"] for i in range(NCORES)], axis=0)
    return out.astype(np.float32)


# revision 14
# speedup vs baseline: 1.8270x; 1.0111x over previous
"""Trainium2 Bass kernel for nn_BagKQMClassModel.

Computation (per batch item b):
    K[b,n,m]   = exp(-d2/(2 s^2)),  d2 = |A[b,n] - C[m]|^2
    out_w[b,m] = (1/N) sum_n comp_w[m] * K^2
    y_w        = out_w / sum_m out_w
    probs      = y_w @ (y_v^2),  y_v = c_y rows normalized

Key transformations used here:
  * K^2 = exp(-d2/s^2), so only one exp per (b,n,m) element is needed.
  * d2 = a2[bn] + b2[m] - 2 g[m,bn] with g = C @ A^T.  Both -a2/2 and -b2/2
    are folded into the matmul by augmenting the contraction dim (K=34):
        row 32: C^T row = 1,       A^T row = -a2/2
        row 33: C^T row = -b2/2,   A^T row = 1
    so one matmul emits g_full with exp argument = (2/s^2) * g_full and the
    activation needs no bias at all.
  * probs = T[:, :10] / T[:, 10] where T[b,:] = sum_n sum_m K2[m,bn]*W[m,:],
    W[m, :10] = comp_w[m] * c_y[m]^2 / |c_y[m]|^2,  W[m, 10] = comp_w[m].
    The 1/N bag weight and normalization cancel.
  * Layout (m on partitions, b*n on free dim): matmul2 contracts m on the PE
    with W as the stationary operand; the final n-reduction happens on the
    tiny (11, bn) result via a segmented DVE reduce.

Sharding: batch 256 -> 32 items per core across 8 cores; c_x/c_y/comp_w
replicated. No collectives (forward only).
"""

import numpy as np

import concourse.bacc as bacc
import concourse.mybir as mybir
import concourse.tile as tile
from concourse.bass import ts
from concourse.bass_utils import run_bass_kernel_spmd
from concourse.masks import make_identity

NCORES = 8
BS, N, DX, DY, M = 256, 128, 32, 10, 2048
BPC = BS // NCORES      # 32 batch items per core
MB = M // 128           # 16 chunks of the component axis
KAUG = DX + 2           # 34: contraction dim with the two folded rows
NBLK = 4                # bn blocks per core
BLKI = BPC // NBLK      # 8 items per block
F_BLK = BLKI * N        # 1024 free elements per (block, m-chunk) tile
MIN_SIGMA = 1e-3
FP32 = mybir.dt.float32
BF16 = mybir.dt.bfloat16
AX = mybir.AxisListType
ALU = mybir.AluOpType
ACTF = mybir.ActivationFunctionType


def _body(tc, inp, cx, cy, cw_d, out_d, scale):
    nc = tc.nc
    from contextlib import ExitStack

    with ExitStack() as ctx:
        const = ctx.enter_context(tc.tile_pool(name="const", bufs=1))
        work = ctx.enter_context(tc.tile_pool(name="work", bufs=2))
        k2p = ctx.enter_context(tc.tile_pool(name="k2p", bufs=3))
        psum = ctx.enter_context(tc.tile_pool(name="psum", bufs=2, space="PSUM"))

        identity = const.tile([128, 128], FP32)
        make_identity(nc, identity)

        # ---- HAM warm-up ---------------------------------------------------
        # The PE clock sits at 1.2 GHz until the activity monitor sees a full
        # ~3.4us window of sustained matmul work; the steady-state here never
        # has >2.5us of contiguous PE time, so without this burst the whole
        # kernel runs at half clock.  ~12 back-to-back dummy matmuls (chained
        # by WAW on one psum tile) run during the input DMAs and trip the
        # monitor once; the later gaps are far too short to re-throttle.
        warm_src = const.tile([128, 512], BF16)
        nc.vector.memset(warm_src, 0.0)
        warm_ps = psum.tile([128, 512], FP32, tag="s", bufs=1)
        warm_w = warm_src[:, 0:128]
        for _ in range(16):
            nc.tensor.matmul(warm_ps, warm_w, warm_src, start=True, stop=True)

        # ---- load replicated constants -----------------------------------
        # c_x goes into an augmented (128, 34) layout per chunk: cols 0:32 are
        # c_x, col 32 = 1.0, col 33 = -b2/2.  Transposing the whole tile then
        # yields the augmented C^T rows without any partition-offset writes.
        cx_aug = const.tile([128, MB, KAUG], FP32)
        nc.sync.dma_start(
            out=cx_aug[:, :, 0:DX], in_=cx.rearrange("(t p) d -> p t d", p=128)
        )
        nc.vector.memset(cx_aug[:, :, DX : DX + 1], 1.0)
        cy_all = const.tile([128, MB, DY], FP32)
        nc.sync.dma_start(out=cy_all, in_=cy.rearrange("(t p) d -> p t d", p=128))
        cw_sb = const.tile([128, MB], FP32)
        nc.sync.dma_start(out=cw_sb, in_=cw_d.rearrange("(t p) -> p t", p=128))

        # ---- load batch shard, leaving room for the two augmented cols ----
        A_aug = const.tile([128, BPC, KAUG], FP32)
        nc.sync.dma_start(
            out=A_aug[:, :, 0:DX], in_=inp.rearrange("t p d -> p t d")
        )
        nc.vector.memset(A_aug[:, :, DX + 1 : DX + 2], 1.0)

        # bf16 for both matmul operands: fp32 matmuls stream at 1/4 PE rate.
        CT = const.tile([KAUG, M], BF16)      # augmented C^T (stationary mm1)
        AT = const.tile([KAUG, BPC, N], BF16)  # augmented A^T (moving mm1)
        W_all = const.tile([128, MB, DY + 1], FP32)
        W_bf = const.tile([128, MB, DY + 1], BF16)
        T_sb = const.tile([DY + 1, BPC], FP32)

        # ---- m-chunk setup: b2 column of cx_aug, then W (all batched) -----
        sqx = work.tile([128, MB, DX], FP32, tag="sqx")
        nc.vector.tensor_mul(sqx, cx_aug[:, :, 0:DX], cx_aug[:, :, 0:DX])
        nc.vector.tensor_reduce(
            out=cx_aug[:, :, DX + 1 : DX + 2], in_=sqx, axis=AX.X, op=ALU.add
        )
        nc.vector.tensor_scalar_mul(
            cx_aug[:, :, DX + 1 : DX + 2], cx_aug[:, :, DX + 1 : DX + 2], -0.5
        )

        sqy = work.tile([128, MB, DY], FP32, tag="sqy")
        nc.vector.tensor_mul(sqy, cy_all, cy_all)
        ssum = work.tile([128, MB], FP32, tag="ssum")
        nc.vector.tensor_reduce(out=ssum, in_=sqy, axis=AX.X, op=ALU.add)
        rec = work.tile([128, MB], FP32, tag="rec")
        nc.vector.reciprocal(rec, ssum)
        facr = work.tile([128, MB], FP32, tag="facr")
        nc.vector.tensor_mul(facr, rec, cw_sb)
        for mb in range(MB):
            nc.vector.tensor_scalar(
                out=W_all[:, mb, 0:DY],
                in0=sqy[:, mb, :],
                scalar1=facr[:, mb : mb + 1],
                scalar2=None,
                op0=ALU.mult,
            )
        nc.vector.tensor_copy(
            W_all[:, :, DY : DY + 1], cw_sb.rearrange("p (t one) -> p t one", one=1)
        )
        nc.vector.tensor_copy(W_bf, W_all)

        # ---- batched -a2/2 column for all items ---------------------------
        sqa = work.tile([128, BPC, DX], FP32, tag="sqa")
        nc.vector.tensor_mul(sqa, A_aug[:, :, 0:DX], A_aug[:, :, 0:DX])
        nc.vector.tensor_reduce(
            out=A_aug[:, :, DX : DX + 1], in_=sqa, axis=AX.X, op=ALU.add
        )
        nc.vector.tensor_scalar_mul(
            A_aug[:, :, DX : DX + 1], A_aug[:, :, DX : DX + 1], -0.5
        )

        # ---- transposes (PE), psum->sbuf casts on the idle ACT engine -----
        for mb in range(MB):
            trc = psum.tile([KAUG, 128], FP32, tag="g", bufs=3)
            nc.tensor.transpose(trc, cx_aug[:, mb, :], identity)
            nc.scalar.copy(CT[:, ts(mb, 128)], trc)
        for i in range(BPC):
            tra = psum.tile([KAUG, 128], FP32, tag="g", bufs=3)
            nc.tensor.transpose(tra, A_aug[:, i, :], identity)
            nc.scalar.copy(AT[:, i, :], tra)



        # ---- main pipeline ------------------------------------------------
        # Software-pipelined: mm2 for step mb-1 is issued after mm1 for step
        # mb, so the PE FIFO never blocks on the ACT output of the current
        # step (otherwise mm1 -> exp -> mm2 serializes each iteration).
        for blk in range(NBLK):
            S = psum.tile([DY + 1, F_BLK], FP32, tag="s", bufs=1)
            k2_prev = None
            for mb in range(MB):
                g = psum.tile([128, F_BLK], FP32, tag="g", bufs=3)
                for q in range(F_BLK // 512):
                    nc.tensor.matmul(
                        g[:, ts(q, 512)],
                        CT[:, ts(mb, 128)],
                        AT[:, blk * BLKI + q * 4 : blk * BLKI + (q + 1) * 4, :],
                        start=True,
                        stop=True,
                    )
                if k2_prev is not None:
                    for q in range(F_BLK // 512):
                        nc.tensor.matmul(
                            S[:, ts(q, 512)],
                            W_bf[:, mb - 1, :],
                            k2_prev[:, ts(q, 512)],
                            start=(mb - 1 == 0),
                            stop=False,
                        )
                K2 = k2p.tile([128, F_BLK], BF16, tag="k2")
                nc.scalar.activation(K2, g, ACTF.Exp, bias=0.0, scale=scale)
                k2_prev = K2
                if blk == 0 and mb == 1:
                    # In-loop HAM warm-up: the pipeline-fill stalls at loop
                    # start re-throttle the PE clock to 1.2 GHz, and the cold
                    # steady state's ~2.4us PE bursts never refill a 3.4us
                    # activity window, so without this the whole loop runs at
                    # ~1.8us/step instead of ~1.0us/step.  12 dependency-free
                    # back-to-back matmuls guarantee one full busy window.
                    warm2 = psum.tile([128, 512], FP32, tag="g", bufs=3)
                    for _ in range(12):
                        nc.tensor.matmul(
                            warm2, CT[:, 0:128], AT[:, 0:4, :], start=True, stop=True
                        )
            for q in range(F_BLK // 512):
                nc.tensor.matmul(
                    S[:, ts(q, 512)],
                    W_bf[:, MB - 1, :],
                    k2_prev[:, ts(q, 512)],
                    start=False,
                    stop=True,
                )
            nc.vector.tensor_reduce(
                out=T_sb[:, blk * BLKI : (blk + 1) * BLKI],
                in_=S.rearrange("p (t n) -> p t n", n=N),
                axis=AX.X,
                op=ALU.add,
            )

        # ---- epilogue: probs = T[:, :10] / T[:, 10] -----------------------
        trT = psum.tile([BPC, DY + 1], FP32, tag="g", bufs=3)
        nc.tensor.transpose(trT, T_sb, identity[0 : DY + 1, 0 : DY + 1])
        Tt = const.tile([BPC, DY + 1], FP32)
        nc.vector.tensor_copy(Tt, trT)
        recd = const.tile([BPC, 1], FP32)
        nc.vector.reciprocal(recd, Tt[:, DY : DY + 1])
        outsb = const.tile([BPC, DY], FP32)
        nc.vector.tensor_scalar(
            out=outsb, in0=Tt[:, 0:DY], scalar1=recd, scalar2=None, op0=ALU.mult
        )
        nc.sync.dma_start(out=out_d, in_=outsb)


def build_program(scale):
    nc = bacc.Bacc(
        "TRN2",
        target_bir_lowering=False,
        debug=False,
        enable_asserts=False,
        num_devices=NCORES,
    )
    inp = nc.dram_tensor("inputs", [BPC, N, DX], FP32, kind="ExternalInput").ap()
    cx = nc.dram_tensor("c_x", [M, DX], FP32, kind="ExternalInput").ap()
    cy = nc.dram_tensor("c_y", [M, DY], FP32, kind="ExternalInput").ap()
    cw = nc.dram_tensor("comp_w", [M], FP32, kind="ExternalInput").ap()
    out = nc.dram_tensor("out", [BPC, DY], FP32, kind="ExternalOutput").ap()
    with tile.TileContext(nc) as tc:
        _body(tc, inp, cx, cy, cw, out, scale)
    nc.compile()
    return nc


_PROGRAM_CACHE: dict = {}


def _get_program(scale):
    nc = _PROGRAM_CACHE.get(scale)
    if nc is None:
        nc = build_program(scale)
        _PROGRAM_CACHE[scale] = nc
    return nc


def make_in_maps(inputs, c_x, c_y, comp_w):
    shards = np.ascontiguousarray(inputs.reshape(NCORES, BPC, N, DX))
    return [
        {
            "inputs": shards[i],
            "c_x": np.ascontiguousarray(c_x),
            "c_y": np.ascontiguousarray(c_y),
            "comp_w": np.ascontiguousarray(comp_w),
        }
        for i in range(NCORES)
    ]


def scale_from_sigma(sigma) -> float:
    s = max(float(np.asarray(sigma, dtype=np.float64)), MIN_SIGMA)
    return float(2.0 / (s * s))


def kernel(inputs, sigma, c_x, c_y, comp_w, _run_kwargs=None):
    nc = _get_program(scale_from_sigma(sigma))
    in_maps = make_in_maps(inputs, c_x, c_y, comp_w)
    res = run_bass_kernel_spmd(
        nc, in_maps, core_ids=list(range(NCORES)), **(_run_kwargs or {})
    )
    out = np.concatenate([res.results[i]["out"] for i in range(NCORES)], axis=0)
    return out.astype(np.float32)
